# revision 17
# baseline (speedup 1.0000x reference)
"""Trainium2 Bass kernel for nn_MultiHeadModel (moe_routing).

Reference computation:
    route  = argmax(x @ W_lab + b_lab, -1)            # [N]
    z      = x @ W_enc + b_enc                        # [N, 64]
    heads  = einsum('nd,ids->nis', z, W_clf) + b_clf  # [N, 8, 4]
    out    = (heads * onehot(route)).reshape(N, 32)

Design:
  1. Encoder+classifier compose into one linear map: heads = x @ W_eff + b_eff
     with W_eff = W_enc @ W_clf_flat (W_clf_flat[d, i*4+s] = W_clf[i, d, s]).
  2. The device streams only xh = fp16(x) (16 MB/core); weights ride the
     moving operand as bf16 double-double folds (W1+W2 = W_lab to 2^-18,
     We1+We2 = W_eff to 2^-18) via 0-step out-AP folds.
  3. Routing tolerance: device logits differ from exact by < ~9e-4
     (dominated by the dropped fp16 residual x-xh). The device widens the
     argmax one-hot to an epsilon-band mask (logit >= max - 2.5e-3), so any
     row whose top-2 gap is within the device's error bound selects >1 head
     group and thereby flags itself in the output. The host detects flagged
     rows (multiple nonzero head groups, ~1% of rows) and recomputes them
     exactly in fp64. Rows with a single selected group are mathematically
     guaranteed to match the exact argmax.
  4. Output is stored as fp16 (4 MB/core) and upcast on host.

HBM traffic per core: 16 MB in + 4 MB out = 20 MB -> ~56 us DMA floor.
PE: one LDWEIGHTS (~105 ns, fixed-cost dominated) + two same-stationary
matmuls per 128-token tile; --enable-ldw-opt=true lets walrus elide the
second LDWEIGHTS.

Layout: host uploads xh pre-transposed (d_in on partitions, tokens on the
free axis, G-grouped column order) so the device does zero transposes.
Per 2048-token macro-tile the 16 matmul pairs write logits into a
[128,16,8] psum tile (one bank) and heads into a [128,16,32] tile (one
bank), so the DVE epilogue is 4 wide ops per macro: reduce_max ->
max-eps -> is_ge mask -> masked multiply (fp16 out).
"""

import sys

if "/opt/trn_rl_repo" not in sys.path:
    sys.path.insert(0, "/opt/trn_rl_repo")

import numpy as np

N_TOTAL = 524288
N_CORES = 8
N_PER_CORE = N_TOTAL // N_CORES  # 65536
D_IN = 128
Y_DIM = 8
S_DIM = 4
D_ENC = 64
OUT_COLS = Y_DIM * S_DIM  # 32

GH = 16                   # tokens per partition per compute group (host layout)
GROUP = 128 * GH          # 2048 tokens per compute group
N_GROUPS = N_PER_CORE // GROUP  # 32
G = 32                    # tokens per partition per DMA macro-tile (2 groups)
MACRO = 128 * G           # 4096 tokens per macro-tile
N_MACROS = N_PER_CORE // MACRO  # 16

EPS = 2.5e-3              # ambiguity band on the routing logits

# moving-operand SBUF layout, all bf16: [W1|W2|We1|We2]
WMOV_COLS = 2 * Y_DIM + 2 * OUT_COLS  # 80

_CACHE = {}

# test.py can read this after calling kernel() to get profile info
LAST_RESULTS = None


def _build(with_bias: bool):
    import concourse.bacc as bacc
    import concourse.bass as bass
    import concourse.mybir as mybir
    import concourse.tile as tile

    f32 = mybir.dt.float32
    f16 = mybir.dt.float16
    bf16 = mybir.dt.bfloat16
    nc = bacc.Bacc("TRN2", target_bir_lowering=False)

    xh_d = nc.dram_tensor("xh", [D_IN, N_PER_CORE], f16, kind="ExternalInput")
    w_d = nc.dram_tensor("w_mov", [D_IN, WMOV_COLS], bf16, kind="ExternalInput")
    if with_bias:
        b_d = nc.dram_tensor(
            "b_big", [1, Y_DIM + OUT_COLS], f32, kind="ExternalInput"
        )
    out_d = nc.dram_tensor("out", [N_PER_CORE, OUT_COLS], f16, kind="ExternalOutput")

    with tile.TileContext(nc) as tc:
        with (
            tc.tile_pool(name="const", bufs=1) as const_pool,
            tc.tile_pool(name="xin", bufs=5) as x_pool,
            tc.tile_pool(name="outs", bufs=4) as out_pool,
            tc.tile_pool(name="small", bufs=4) as small_pool,
            tc.tile_pool(name="lgp", bufs=2, space=bass.MemorySpace.PSUM) as lg_pool,
            tc.tile_pool(name="hdp", bufs=2, space=bass.MemorySpace.PSUM) as hd_pool,
        ):
            w_sb = const_pool.tile([D_IN, WMOV_COLS], bf16)
            nc.sync.dma_start(w_sb[:], w_d[:])

            if with_bias:
                ones_sb = const_pool.tile([1, 128], f32)
                nc.gpsimd.memset(ones_sb[:], 1.0)
                b_row = const_pool.tile([1, Y_DIM + OUT_COLS], f32)
                nc.sync.dma_start(b_row[:], b_d[:])
                with tc.tile_pool(
                    name="biasp", bufs=1, space=bass.MemorySpace.PSUM
                ) as biasp_pool:
                    bias_ps = biasp_pool.tile([128, Y_DIM + OUT_COLS], f32)
                    nc.tensor.matmul(bias_ps[:], ones_sb[:], b_row[:])
                    bias_sb = const_pool.tile([128, Y_DIM + OUT_COLS], f32)
                    nc.scalar.copy(bias_sb[:], bias_ps[:])

            for m in range(N_MACROS):
                r0 = m * MACRO
                xh_sb = x_pool.tile([D_IN, MACRO], f16)
                nc.sync.dma_start(xh_sb[:], xh_d[:, r0 : r0 + MACRO])
                out_sb = out_pool.tile([128, G, OUT_COLS], f16)

                H = G // 2  # 16 tokens/partition per compute group
                for half in range(2):
                    lg_ps = lg_pool.tile([128, H, Y_DIM], f32, name=f"lg{half}")
                    hd_ps = hd_pool.tile(
                        [128, H, OUT_COLS], f32, name=f"hd{half}"
                    )
                    for q in range(H):
                        t = half * H + q
                        hs = xh_sb[:, t * 128 : (t + 1) * 128]
                        # logits: psum[:, q, 0:8] = xh @ (W1 + W2)
                        nc.tensor.matmul(
                            lg_ps[:, q, :][:, None, :].broadcast_to(
                                [128, 2, Y_DIM]
                            ),
                            hs,
                            w_sb[:, 0 : 2 * Y_DIM],
                            start=True,
                            stop=True,
                            skip_group_check=True,
                        )
                        # heads: psum[:, q, 0:32] = xh @ (We1 + We2)
                        nc.tensor.matmul(
                            hd_ps[:, q, :][:, None, :].broadcast_to(
                                [128, 2, OUT_COLS]
                            ),
                            hs,
                            w_sb[:, 2 * Y_DIM : WMOV_COLS],
                            start=True,
                            stop=True,
                            skip_group_check=True,
                        )

                    if with_bias:
                        nc.vector.tensor_tensor(
                            lg_ps[:],
                            lg_ps[:],
                            bias_sb[:, 0:Y_DIM][:, None, :].broadcast_to(
                                [128, H, Y_DIM]
                            ),
                            mybir.AluOpType.add,
                        )
                        nc.vector.tensor_tensor(
                            hd_ps[:],
                            hd_ps[:],
                            bias_sb[:, Y_DIM:][:, None, :].broadcast_to(
                                [128, H, OUT_COLS]
                            ),
                            mybir.AluOpType.add,
                        )

                    maxl = small_pool.tile([128, H], f32, name=f"maxl{half}")
                    nc.vector.tensor_reduce(
                        maxl[:],
                        lg_ps[:],
                        axis=mybir.AxisListType.X,
                        op=mybir.AluOpType.max,
                    )
                    nc.vector.tensor_scalar_sub(maxl[:], maxl[:], EPS)
                    mask = small_pool.tile(
                        [128, H, Y_DIM], f32, name=f"mask{half}"
                    )
                    nc.vector.tensor_tensor(
                        mask[:],
                        lg_ps[:],
                        maxl[:][:, :, None].broadcast_to([128, H, Y_DIM]),
                        mybir.AluOpType.is_ge,
                    )
                    nc.vector.tensor_tensor(
                        out_sb[:, half * H : (half + 1) * H, :].rearrange(
                            "p g (i s) -> p g i s", s=S_DIM
                        ),
                        hd_ps[:].rearrange("p g (i s) -> p g i s", s=S_DIM),
                        mask[:][:, :, :, None].broadcast_to(
                            [128, H, Y_DIM, S_DIM]
                        ),
                        mybir.AluOpType.mult,
                    )

                # one 256KB store per macro (fewer HBM read/write turnarounds)
                # on the ACT HWDGE ring so its DVE-wait can't head-of-line-
                # block the prefetch loads on the sync ring
                nc.scalar.dma_start(
                    out_d[r0 : r0 + MACRO, :].rearrange(
                        "(h p g) j -> p h g j", h=2, p=128
                    ),
                    out_sb[:],
                )

    nc.compile()
    return nc


def _get_nc(with_bias: bool):
    key = ("nc", with_bias)
    if key not in _CACHE:
        _CACHE[key] = _build(with_bias)
    return _CACHE[key]


def _host_transpose_shard(xs):
    """[65536, 128] -> [128, 65536] with GH-grouped column order.

    Device column (g, t*128 + p) must hold token g*GROUP + p*GH + t so that
    the PSUM/output partition p covers GH consecutive tokens per group.
    """
    xs4 = xs.reshape(N_GROUPS, 128, GH, D_IN)  # [g, p, t, d]
    return np.ascontiguousarray(
        xs4.transpose(3, 0, 2, 1).reshape(D_IN, N_PER_CORE)
    )


def kernel(x, W_lab, b_lab, W_enc, b_enc, W_clf, b_clf):
    global LAST_RESULTS
    from concourse.bass_utils import run_bass_kernel_spmd

    x = np.asarray(x, dtype=np.float32)
    W_lab = np.asarray(W_lab, dtype=np.float32)
    b_lab = np.asarray(b_lab, dtype=np.float32)
    W_enc = np.asarray(W_enc, dtype=np.float32)
    b_enc = np.asarray(b_enc, dtype=np.float32)
    W_clf = np.asarray(W_clf, dtype=np.float32)
    b_clf = np.asarray(b_clf, dtype=np.float32)

    # Fold encoder + classifier into one [128, 32] map (all linear).
    w_clf_flat = np.transpose(W_clf, (1, 0, 2)).reshape(D_ENC, OUT_COLS)
    w_eff = (W_enc.astype(np.float64) @ w_clf_flat.astype(np.float64)).astype(
        np.float32
    )
    b_eff = (
        b_enc.astype(np.float64) @ w_clf_flat.astype(np.float64)
        + b_clf.reshape(OUT_COLS).astype(np.float64)
    ).astype(np.float32)
    b_big = np.concatenate([b_lab, b_eff]).astype(np.float32)  # [40]

    import ml_dtypes

    bf = ml_dtypes.bfloat16
    xh = x.astype(np.float16)

    def bf16_double(w):
        w1 = w.astype(bf)
        w2 = (w - w1.astype(np.float32)).astype(bf)
        return w1, w2

    w1, w2 = bf16_double(W_lab)
    we1, we2 = bf16_double(w_eff)
    w_mov = np.ascontiguousarray(
        np.concatenate([w1, w2, we1, we2], axis=1).astype(bf)
    )  # [128, 80] bf16

    with_bias = bool(np.any(b_big != 0.0))
    nc = _get_nc(with_bias)

    in_maps = []
    for i in range(N_CORES):
        sl = slice(i * N_PER_CORE, (i + 1) * N_PER_CORE)
        m = {
            "xh": _host_transpose_shard(xh[sl]),
            "w_mov": w_mov,
        }
        if with_bias:
            m["b_big"] = b_big.reshape(1, Y_DIM + OUT_COLS)
        in_maps.append(m)

    # The container default passes --enable-ldw-opt=false to the neuronxcc
    # backend; this kernel issues two matmuls per stationary tile, so the
    # LDWEIGHTS-elision pass halves its PE critical path.
    from concourse import compiler_utils

    old_flags = compiler_utils.get_compiler_flags()
    new_flags = [
        f.replace("--enable-ldw-opt=false", "--enable-ldw-opt=true")
        for f in old_flags
    ]
    try:
        compiler_utils.set_compiler_flags(new_flags)
        res = run_bass_kernel_spmd(nc, in_maps, list(range(N_CORES)))
    finally:
        compiler_utils.set_compiler_flags(old_flags)
    LAST_RESULTS = res
    out = np.concatenate(
        [res.results[i]["out"] for i in range(N_CORES)], axis=0
    ).astype(np.float32)

    # Host-side exact fix-up of ambiguity-flagged rows: any row whose
    # epsilon-band mask selected != 1 head group.
    nz = (out.reshape(N_TOTAL, Y_DIM, S_DIM) != 0.0).any(axis=2)
    amb = nz.sum(axis=1) != 1
    idx = np.nonzero(amb)[0]
    if idx.size:
        xi = x[idx].astype(np.float64)
        lg = xi @ W_lab.astype(np.float64) + b_lab.astype(np.float64)
        route = np.argmax(lg, axis=1)
        heads = xi @ w_eff.astype(np.float64) + b_eff.astype(np.float64)
        patch = np.zeros((idx.size, Y_DIM, S_DIM), dtype=np.float32)
        rows = np.arange(idx.size)
        vals = heads.reshape(idx.size, Y_DIM, S_DIM)[rows, route, :]
        patch[rows, route, :] = vals.astype(np.float32)
        out[idx] = patch.reshape(idx.size, OUT_COLS)
    return out


# revision 19
# speedup vs baseline: 1.0555x; 1.0555x over previous
"""Trainium2 Bass kernel for nn_MultiHeadModel (moe_routing).

Reference computation:
    route  = argmax(x @ W_lab + b_lab, -1)            # [N]
    z      = x @ W_enc + b_enc                        # [N, 64]
    heads  = einsum('nd,ids->nis', z, W_clf) + b_clf  # [N, 8, 4]
    out    = (heads * onehot(route)).reshape(N, 32)

Design:
  1. Encoder+classifier compose into one linear map: heads = x @ W_eff + b_eff
     with W_eff = W_enc @ W_clf_flat (W_clf_flat[d, i*4+s] = W_clf[i, d, s]).
  2. The device streams only xh = fp16(x) (16 MB/core); weights ride the
     moving operand as bf16 double-double folds (W1+W2 = W_lab to 2^-18,
     We1+We2 = W_eff to 2^-18) via 0-step out-AP folds.
  3. Routing tolerance: device logits differ from exact by < ~9e-4
     (dominated by the dropped fp16 residual x-xh). The device widens the
     argmax one-hot to an epsilon-band mask (logit >= max - 2.5e-3), so any
     row whose top-2 gap is within the device's error bound selects >1 head
     group and thereby flags itself in the output. The host detects flagged
     rows (multiple nonzero head groups, ~0.7% of rows) and recomputes them
     exactly in fp64. Rows with a single selected group are mathematically
     guaranteed to match the exact argmax (eps > 2x the logit error bound).
  4. Output is stored as fp16 (4 MB/core) and upcast on host.

HBM traffic per core: 16 MB in + 4 MB out = 20 MB; the kernel runs at the
DMA roofline (~310 GB/s effective end-to-end incl. read/write turnaround).

Layout: host uploads xh pre-transposed (d_in on partitions, tokens on the
free axis, 16-token-grouped column order) so the device does zero
transposes. Per 2048-token macro: 512KB load (sync ring), 16 tiles x
(logits MM 2-fold 16 cols + heads MM 2-fold 64 cols, shared stationary ->
LDWEIGHTS hides under the matmul stream) into psum tiles lg [128,16,8] /
hd [128,16,32] (one bank each), then a 4-op DVE epilogue: reduce_max ->
max-eps -> is_ge mask -> masked multiply (fp16 out), 128KB store
(scalar ring).
"""

import sys

if "/opt/trn_rl_repo" not in sys.path:
    sys.path.insert(0, "/opt/trn_rl_repo")

import numpy as np

N_TOTAL = 524288
N_CORES = 8
N_PER_CORE = N_TOTAL // N_CORES  # 65536
D_IN = 128
Y_DIM = 8
S_DIM = 4
D_ENC = 64
OUT_COLS = Y_DIM * S_DIM  # 32

G = 16                    # tokens per partition per macro-tile
MACRO = 128 * G           # 2048 tokens per macro-tile
N_MACROS = N_PER_CORE // MACRO  # 32

EPS = 2.5e-3              # ambiguity band on the routing logits

# moving-operand SBUF layout, all bf16: [W1|W2|We1|We2]
WMOV_COLS = 2 * Y_DIM + 2 * OUT_COLS  # 80

_CACHE = {}

# test.py can read this after calling kernel() to get profile info
LAST_RESULTS = None


def _build(with_bias: bool):
    import concourse.bacc as bacc
    import concourse.bass as bass
    import concourse.mybir as mybir
    import concourse.tile as tile

    f32 = mybir.dt.float32
    f16 = mybir.dt.float16
    bf16 = mybir.dt.bfloat16
    nc = bacc.Bacc("TRN2", target_bir_lowering=False)

    xh_d = nc.dram_tensor("xh", [D_IN, N_PER_CORE], f16, kind="ExternalInput")
    w_d = nc.dram_tensor("w_mov", [D_IN, WMOV_COLS], bf16, kind="ExternalInput")
    if with_bias:
        b_d = nc.dram_tensor(
            "b_big", [1, Y_DIM + OUT_COLS], f32, kind="ExternalInput"
        )
    out_d = nc.dram_tensor("out", [N_PER_CORE, OUT_COLS], f16, kind="ExternalOutput")

    with tile.TileContext(nc) as tc:
        with (
            tc.tile_pool(name="const", bufs=1) as const_pool,
            tc.tile_pool(name="xin", bufs=8) as x_pool,
            tc.tile_pool(name="outs", bufs=6) as out_pool,
            tc.tile_pool(name="small", bufs=4) as small_pool,
            tc.tile_pool(name="lgp", bufs=3, space=bass.MemorySpace.PSUM) as lg_pool,
            tc.tile_pool(name="hdp", bufs=3, space=bass.MemorySpace.PSUM) as hd_pool,
        ):
            w_sb = const_pool.tile([D_IN, WMOV_COLS], bf16)
            nc.sync.dma_start(w_sb[:], w_d[:])

            if with_bias:
                ones_sb = const_pool.tile([1, 128], f32)
                nc.gpsimd.memset(ones_sb[:], 1.0)
                b_row = const_pool.tile([1, Y_DIM + OUT_COLS], f32)
                nc.sync.dma_start(b_row[:], b_d[:])
                with tc.tile_pool(
                    name="biasp", bufs=1, space=bass.MemorySpace.PSUM
                ) as biasp_pool:
                    bias_ps = biasp_pool.tile([128, Y_DIM + OUT_COLS], f32)
                    nc.tensor.matmul(bias_ps[:], ones_sb[:], b_row[:])
                    bias_sb = const_pool.tile([128, Y_DIM + OUT_COLS], f32)
                    nc.scalar.copy(bias_sb[:], bias_ps[:])

            for m in range(N_MACROS):
                r0 = m * MACRO
                xh_sb = x_pool.tile([D_IN, MACRO], f16, name="xh")
                nc.sync.dma_start(xh_sb[:], xh_d[:, r0 : r0 + MACRO])
                out_sb = out_pool.tile([128, G, OUT_COLS], f16, name="osb")

                lg_ps = lg_pool.tile([128, G, Y_DIM], f32, name="lg")
                hd_ps = hd_pool.tile([128, G, OUT_COLS], f32, name="hd")
                for t in range(G):
                    hs = xh_sb[:, t * 128 : (t + 1) * 128]
                    # logits: psum[:, t, 0:8] = xh @ (W1 + W2)
                    nc.tensor.matmul(
                        lg_ps[:, t, :][:, None, :].broadcast_to([128, 2, Y_DIM]),
                        hs,
                        w_sb[:, 0 : 2 * Y_DIM],
                        start=True,
                        stop=True,
                        skip_group_check=True,
                    )
                    # heads: psum[:, t, 0:32] = xh @ (We1 + We2)
                    nc.tensor.matmul(
                        hd_ps[:, t, :][:, None, :].broadcast_to(
                            [128, 2, OUT_COLS]
                        ),
                        hs,
                        w_sb[:, 2 * Y_DIM : WMOV_COLS],
                        start=True,
                        stop=True,
                        skip_group_check=True,
                    )

                if with_bias:
                    nc.vector.tensor_tensor(
                        lg_ps[:],
                        lg_ps[:],
                        bias_sb[:, 0:Y_DIM][:, None, :].broadcast_to(
                            [128, G, Y_DIM]
                        ),
                        mybir.AluOpType.add,
                    )
                    nc.vector.tensor_tensor(
                        hd_ps[:],
                        hd_ps[:],
                        bias_sb[:, Y_DIM:][:, None, :].broadcast_to(
                            [128, G, OUT_COLS]
                        ),
                        mybir.AluOpType.add,
                    )

                maxl = small_pool.tile([128, G], f32, name="maxl")
                nc.vector.tensor_reduce(
                    maxl[:],
                    lg_ps[:],
                    axis=mybir.AxisListType.X,
                    op=mybir.AluOpType.max,
                )
                nc.vector.tensor_scalar_sub(maxl[:], maxl[:], EPS)
                mask = small_pool.tile([128, G, Y_DIM], f32, name="mask")
                nc.vector.tensor_tensor(
                    mask[:],
                    lg_ps[:],
                    maxl[:][:, :, None].broadcast_to([128, G, Y_DIM]),
                    mybir.AluOpType.is_ge,
                )
                nc.vector.tensor_tensor(
                    out_sb[:].rearrange("p g (i s) -> p g i s", s=S_DIM),
                    hd_ps[:].rearrange("p g (i s) -> p g i s", s=S_DIM),
                    mask[:][:, :, :, None].broadcast_to(
                        [128, G, Y_DIM, S_DIM]
                    ),
                    mybir.AluOpType.mult,
                )

                # stores ride the ACT HWDGE ring so their DVE-wait can't
                # head-of-line-block the prefetch loads on the sync ring
                nc.scalar.dma_start(
                    out_d[r0 : r0 + MACRO, :].rearrange("(p g) j -> p (g j)", p=128),
                    out_sb[:],
                )

    nc.compile()
    return nc


def _get_nc(with_bias: bool):
    key = ("nc", with_bias)
    if key not in _CACHE:
        _CACHE[key] = _build(with_bias)
    return _CACHE[key]


def _host_transpose_shard(xs):
    """[65536, 128] -> [128, 65536] with G-grouped column order.

    Device column (m, t*128 + p) must hold token m*MACRO + p*G + t so that
    the PSUM/output partition p covers G consecutive tokens per macro.
    """
    xs4 = xs.reshape(N_MACROS, 128, G, D_IN)  # [m, p, t, d]
    return np.ascontiguousarray(
        xs4.transpose(3, 0, 2, 1).reshape(D_IN, N_PER_CORE)
    )


def kernel(x, W_lab, b_lab, W_enc, b_enc, W_clf, b_clf):
    global LAST_RESULTS
    from concourse.bass_utils import run_bass_kernel_spmd

    x = np.asarray(x, dtype=np.float32)
    W_lab = np.asarray(W_lab, dtype=np.float32)
    b_lab = np.asarray(b_lab, dtype=np.float32)
    W_enc = np.asarray(W_enc, dtype=np.float32)
    b_enc = np.asarray(b_enc, dtype=np.float32)
    W_clf = np.asarray(W_clf, dtype=np.float32)
    b_clf = np.asarray(b_clf, dtype=np.float32)

    # Fold encoder + classifier into one [128, 32] map (all linear).
    w_clf_flat = np.transpose(W_clf, (1, 0, 2)).reshape(D_ENC, OUT_COLS)
    w_eff = (W_enc.astype(np.float64) @ w_clf_flat.astype(np.float64)).astype(
        np.float32
    )
    b_eff = (
        b_enc.astype(np.float64) @ w_clf_flat.astype(np.float64)
        + b_clf.reshape(OUT_COLS).astype(np.float64)
    ).astype(np.float32)
    b_big = np.concatenate([b_lab, b_eff]).astype(np.float32)  # [40]

    import ml_dtypes

    bf = ml_dtypes.bfloat16
    xh = x.astype(np.float16)

    def bf16_double(w):
        w1 = w.astype(bf)
        w2 = (w - w1.astype(np.float32)).astype(bf)
        return w1, w2

    w1, w2 = bf16_double(W_lab)
    we1, we2 = bf16_double(w_eff)
    w_mov = np.ascontiguousarray(
        np.concatenate([w1, w2, we1, we2], axis=1).astype(bf)
    )  # [128, 80] bf16

    with_bias = bool(np.any(b_big != 0.0))
    nc = _get_nc(with_bias)

    in_maps = []
    for i in range(N_CORES):
        sl = slice(i * N_PER_CORE, (i + 1) * N_PER_CORE)
        m = {
            "xh": _host_transpose_shard(xh[sl]),
            "w_mov": w_mov,
        }
        if with_bias:
            m["b_big"] = b_big.reshape(1, Y_DIM + OUT_COLS)
        in_maps.append(m)

    # The container default passes --enable-ldw-opt=false to the neuronxcc
    # backend; flip it so walrus can elide/overlap the per-tile LDWEIGHTS.
    from concourse import compiler_utils

    old_flags = compiler_utils.get_compiler_flags()
    new_flags = [
        f.replace("--enable-ldw-opt=false", "--enable-ldw-opt=true")
        for f in old_flags
    ]
    try:
        compiler_utils.set_compiler_flags(new_flags)
        res = run_bass_kernel_spmd(nc, in_maps, list(range(N_CORES)))
    finally:
        compiler_utils.set_compiler_flags(old_flags)
    LAST_RESULTS = res
    out = np.concatenate(
        [res.results[i]["out"] for i in range(N_CORES)], axis=0
    ).astype(np.float32)

    # Host-side exact fix-up of ambiguity-flagged rows: any row whose
    # epsilon-band mask selected != 1 head group.
    nz = (out.reshape(N_TOTAL, Y_DIM, S_DIM) != 0.0).any(axis=2)
    amb = nz.sum(axis=1) != 1
    idx = np.nonzero(amb)[0]
    if idx.size:
        xi = x[idx].astype(np.float64)
        lg = xi @ W_lab.astype(np.float64) + b_lab.astype(np.float64)
        route = np.argmax(lg, axis=1)
        heads = xi @ w_eff.astype(np.float64) + b_eff.astype(np.float64)
        patch = np.zeros((idx.size, Y_DIM, S_DIM), dtype=np.float32)
        rows = np.arange(idx.size)
        vals = heads.reshape(idx.size, Y_DIM, S_DIM)[rows, route, :]
        patch[rows, route, :] = vals.astype(np.float32)
        out[idx] = patch.reshape(idx.size, OUT_COLS)
    return out
